# revision 22
# baseline (speedup 1.0000x reference)
"""Trainium2 Bass kernel for a chain of 2 invertible-ResNet blocks
(dense MLP 2->256, 4x 256->256, 256->2, ELU, residual) over 1M points.

Strategy: pure data parallel over 8 NeuronCores; points transposed to
[2, N] on host so activations live as [256, FD] tiles (features on
partitions, points on the free dim).  Matmuls run in float32r (full PE
rate).  ELU is computed in 2 instructions per tile:
    ACT:  e  = Exp(y + b_eff)          (PSUM -> SBUF, bias fused)
    DVE:  h'' = max(y, -b_eff) + min(e, 1)   (one custom fused op)
with the resulting constant shift (b_eff - 1) folded into the next
layer's effective bias (precomputed on host).

v3: FD=1024 two-PSUM-bank m-tile units (one ACT + one DVE instruction
per [128, 1024] m-tile halves the per-op fixed overhead on the two
bottleneck engines; DVE at 0.96 GHz 1x is the binding engine at
~23.8 us busy per 1024-point chunk, ACT ~23.2, PE ~18.1) + block-
granular software pipelining: stream B runs one iResNet block (5 ELU
layers) behind stream A, layer-interleaved in emission order, so the
serial block boundary (out-proj matmuls -> ACT evacuation -> next
block's input matmuls) of either stream is always covered by ~5 layers
of dense MM/ACT/DVE work from the other stream.  Step order:
    zip(A.block0(c), B.block1(c-1)) ; zip(A.block1(c), B.block0(c+1))
x1 state lives in persistent SBUF tiles carried across For_i
iterations; prologue/epilogue peel the pipeline edges.  usteps=7 steps
(14 chunks, ~1900 instructions) per For_i iteration balances the
per-iteration all-engine barrier (~7.5 us) against sequencer
instruction-fetch pressure (bodies over ~2500 instructions degrade;
fully unrolled is ~4% slower).
"""

import numpy as np

import concourse.bass as bass
import concourse.tile as tile
from concourse import bacc, mybir
from concourse.bass_utils import run_bass_kernel_spmd
from concourse.dve_spec import Spec, Src0, Src1, C0, C1, maxx, minn
import concourse.dve_ops as dve_ops
from concourse.dve_ops import DveOp

F32 = mybir.dt.float32
F32R = mybir.dt.float32r

NUM_NODES = 2
H = 256
L = 4
D = 2
N_CORES = 8

FD = 1024          # points per chunk (free dim; m-tile = 2 PSUM banks)
HB = 512           # matmul column-half (one PSUM bank)


def _register_elu_tail():
    name = "ELU_TAIL_ANT"
    for op in dve_ops.OPS:
        if op.name == name:
            return op
    op = DveOp(
        name,
        Spec(
            body=maxx(Src0, C0) + minn(Src1, C1),
            reference=lambda in0, in1, s0, s1, imm2: (
                np.maximum(in0.astype(np.float32), s0)
                + np.minimum(in1.astype(np.float32), s1)
            ),
        ),
        subdim=False,
        uops_sha={"v3": "b9e41bc1a54edf6f", "v4": "2155f01abd9df135"},
    )
    dve_ops.OPS.append(op)
    dve_ops._SUB_OPCODE_FOR_NAME[name] = (
        dve_ops._CUSTOM_DVE_ROW_BASE + len(dve_ops.OPS) - 1
    )
    dve_ops.CUSTOM_DVE_SPECS[name] = op.spec
    return op


def _effective_params(w_in, b_in, w_hid, b_hid, w_out, b_out):
    """Fold the ELU-tail constant shifts into effective biases (float64).

    Unfused form: x1 is materialized, so block1's input bias is the true
    b_in[1]; bo0 rides the x1 evacuation and bo1 the final output.
    """
    w_hid = w_hid.astype(np.float64)
    b_hid = b_hid.astype(np.float64)
    w_out = w_out.astype(np.float64)
    b_out = b_out.astype(np.float64)
    b_in = b_in.astype(np.float64)

    b_eff = np.zeros((2 * (1 + L), H))          # per ELU layer
    b_eff[0] = b_in[0]
    c = b_eff[0] - 1.0
    for l in range(L):
        b_eff[1 + l] = b_hid[0, l] + c @ w_hid[0, l]
        c = b_eff[1 + l] - 1.0
    bo0 = b_out[0] + c @ w_out[0]               # [2] -> x1 evac bias
    b_eff[5] = b_in[1]
    c = b_eff[5] - 1.0
    for l in range(L):
        b_eff[6 + l] = b_hid[1, l] + c @ w_hid[1, l]
        c = b_eff[6 + l] - 1.0
    bo1 = b_out[1] + c @ w_out[1]               # [2] -> final out bias

    bp = np.zeros((128, 20), np.float32)
    bn = np.zeros((128, 20), np.float32)
    for j in range(10):
        for m in range(2):
            col = b_eff[j, m * 128:(m + 1) * 128]
            bp[:, j * 2 + m] = col.astype(np.float32)
            bn[:, j * 2 + m] = (-col).astype(np.float32)
    bo = np.stack([bo0, bo1], axis=1).astype(np.float32)   # [2, 2]
    return bp, bn, bo


def _build_program(nsh, usteps, n_iters, repeat=1):
    """Build the SPMD Bass program for one core processing `nsh` points.

    nsh = (1 + usteps * n_iters) * 2 * FD  total chunks; each step is a
    pair of chunks (stream A even, stream B odd).  When n_iters > 1 a
    hardware For_i loop runs `usteps` steps per iteration; step 0 and
    the last B.block1 are peeled into prologue/epilogue.  `repeat`
    re-runs the whole pass (benchmarking aid; output identical).
    """
    ELU_TAIL = _register_elu_tail()
    nc = bacc.Bacc("TRN2", target_bir_lowering=False, debug=False,
                   num_devices=N_CORES)

    uvT = nc.declare_dram_parameter("uvT", [D, nsh], F32, isOutput=False).ap()
    WIN = nc.declare_dram_parameter("WIN", [2, D, H], F32, isOutput=False).ap()
    WH = nc.declare_dram_parameter("WH", [8, H, H], F32, isOutput=False).ap()
    WO = nc.declare_dram_parameter("WO", [2, H, D], F32, isOutput=False).ap()
    IDE = nc.declare_dram_parameter("IDE", [D, D], F32, isOutput=False).ap()
    BP = nc.declare_dram_parameter("BP", [128, 20], F32, isOutput=False).ap()
    BN = nc.declare_dram_parameter("BN", [128, 20], F32, isOutput=False).ap()
    BO = nc.declare_dram_parameter("BO", [D, 2], F32, isOutput=False).ap()
    outT = nc.declare_dram_parameter("outT", [D, nsh], F32, isOutput=True).ap()

    with tile.TileContext(nc) as tc:
        with (
            tc.tile_pool(name="wpool", bufs=1) as wp,
            tc.tile_pool(name="xpool", bufs=3) as xp,
            tc.tile_pool(name="epool", bufs=3) as ep,
            tc.tile_pool(name="hpool", bufs=3) as hp,
            tc.tile_pool(name="opool", bufs=2) as op,
            tc.tile_pool(name="ypool", bufs=2, space="PSUM") as yp,
        ):
            # ---- persistent weights/biases (loaded once) ----
            win = [wp.tile([D, H], F32R, tag=f"win{i}", name=f"win{i}") for i in range(2)]
            for i in range(2):
                nc.gpsimd.dma_start(out=win[i], in_=WIN[i])
            wh = [[wp.tile([128, H], F32R, tag=f"wh{j}k{k}", name=f"wh{j}k{k}") for k in range(2)]
                  for j in range(8)]
            for j in range(8):
                for k in range(2):
                    nc.gpsimd.dma_start(out=wh[j][k],
                                        in_=WH[j, k * 128:(k + 1) * 128, :])
            wo = [[wp.tile([128, D], F32R, tag=f"wo{i}k{k}", name=f"wo{i}k{k}") for k in range(2)]
                  for i in range(2)]
            for i in range(2):
                for k in range(2):
                    nc.gpsimd.dma_start(out=wo[i][k],
                                        in_=WO[i, k * 128:(k + 1) * 128, :])
            ide = wp.tile([D, D], F32R, tag="ide")
            nc.gpsimd.dma_start(out=ide, in_=IDE)
            bp = wp.tile([128, 20], F32, tag="bp")
            nc.gpsimd.dma_start(out=bp, in_=BP)
            bn = wp.tile([128, 20], F32, tag="bn")
            nc.gpsimd.dma_start(out=bn, in_=BN)
            bo = wp.tile([D, 2], F32, tag="bo")
            nc.gpsimd.dma_start(out=bo, in_=BO)
            # persistent x1 state per stream (carried across steps)
            x1 = [wp.tile([D, FD], F32R, tag=f"x1s{s}", name=f"x1s{s}")
                  for s in range(2)]

            def load_x0(s, sl):
                x0 = xp.tile([D, FD], F32R, name=f"x0s{s}", tag=f"x0s{s}")
                nc.gpsimd.dma_start(out=x0, in_=uvT[:, sl])
                return x0

            def block_items(s, i, xin, out_sl=None):
                """Emission items (one per layer + out-proj) for block i of
                stream s.  Returned closures are called in order, possibly
                interleaved with another stream's items.

                xin: [D, FD] SBUF input (x0 for block0, x1 for block1).
                block0 evacuates into the persistent x1[s]; block1 into a
                fresh xo tile, DMA'd to outT[:, out_sl]."""
                st = {"h": [None, None]}

                def layer_item(jj):
                    j = 5 * i + jj
                    h = st["h"]
                    newh = [None, None]
                    for m in range(2):
                        mcs = slice(m * 128, (m + 1) * 128)
                        y = yp.tile([128, FD], F32, name=f"ys{s}m{m}",
                                    tag=f"ys{s}")
                        for hf in range(2):
                            cs = slice(hf * HB, (hf + 1) * HB)
                            if jj == 0:
                                nc.tensor.matmul(y[:, cs], win[i][:, mcs],
                                                 xin[:, cs],
                                                 start=True, stop=True)
                            else:
                                jh = 4 * i + jj - 1
                                nc.tensor.matmul(y[:, cs], wh[jh][0][:, mcs],
                                                 h[0][:, cs],
                                                 start=True, stop=False)
                                nc.tensor.matmul(y[:, cs], wh[jh][1][:, mcs],
                                                 h[1][:, cs],
                                                 start=False, stop=True)
                        col = j * 2 + m
                        e = ep.tile([128, FD], F32, name=f"es{s}m{m}",
                                    tag=f"es{s}m{m}")
                        nc.scalar.activation(
                            e, y, mybir.ActivationFunctionType.Exp,
                            bias=bp[:, col:col + 1])
                        hn = hp.tile([128, FD], F32R, name=f"hs{s}m{m}",
                                     tag=f"hs{s}m{m}")
                        nc.vector._custom_dve(ELU_TAIL, out=hn, in0=y, in1=e,
                                              s0=bn[:, col:col + 1], s1=1.0)
                        newh[m] = hn
                    st["h"] = newh

                def out_item():
                    h = st["h"]
                    if i == 0:
                        dst = x1[s]
                    else:
                        dst = op.tile([D, FD], F32, name=f"xos{s}",
                                      tag=f"xos{s}")
                    for hf in range(2):
                        cs = slice(hf * HB, (hf + 1) * HB)
                        yo = yp.tile([D, HB], F32, name=f"yo{i}s{s}h{hf}",
                                     tag=f"ys{s}")
                        nc.tensor.matmul(yo, ide, xin[:, cs],
                                         start=True, stop=False)
                        nc.tensor.matmul(yo, wo[i][0], h[0][:, cs],
                                         start=False, stop=False)
                        nc.tensor.matmul(yo, wo[i][1], h[1][:, cs],
                                         start=False, stop=True)
                        nc.scalar.activation(
                            dst[:, cs], yo,
                            mybir.ActivationFunctionType.Identity,
                            bias=bo[:, i:i + 1])
                    if i == 1:
                        nc.sync.dma_start(out=outT[:, out_sl], in_=dst)

                return [lambda jj=jj: layer_item(jj) for jj in range(5)] + \
                    [out_item]

            def mk(base):
                return (slice(base, base + FD) if isinstance(base, int)
                        else bass.ds(base, FD))

            def interleave(xs, ys):
                for t in range(max(len(xs), len(ys or []))):
                    if t < len(xs):
                        xs[t]()
                    if ys and t < len(ys):
                        ys[t]()

            def step(abase, first=False):
                """One pipeline step.  abase = A-chunk base offset (int or
                register expr); B of this step is abase + FD; B's block1 of
                the PREVIOUS step (chunk abase - FD) is emitted here,
                layer-interleaved with A's, so B lags A by one block and
                every block boundary is covered by the other stream."""
                asl = mk(abase)
                x0a = load_x0(0, asl)
                x0b = load_x0(1, mk(abase + FD))
                bi = (None if first
                      else block_items(1, 1, x1[1], mk(abase - FD)))
                interleave(block_items(0, 0, x0a), bi)     # A.b0 || B.b1'
                interleave(block_items(0, 1, x1[0], asl),  # A.b1 || B.b0
                           block_items(1, 0, x0b))

            n_steps = nsh // (2 * FD)
            assert n_steps == 1 + usteps * n_iters, (nsh, usteps, n_iters)
            for _rep in range(repeat):
                # prologue: step 0 (no lagged B.block1 yet)
                step(0, first=True)
                if n_iters == 1:
                    for st in range(1, n_steps):
                        step(st * 2 * FD)
                else:
                    stride = 2 * FD * usteps
                    with tc.For_i(2 * FD, 2 * FD + n_iters * stride, stride,
                                  hint_engines=(mybir.EngineType.PE,)) as it:
                        for t in range(usteps):
                            step(it + t * 2 * FD)
                # epilogue: last B.block1
                interleave(block_items(
                    1, 1, x1[1],
                    slice((2 * n_steps - 1) * FD, 2 * n_steps * FD)), None)

    nc.finalize()
    return nc


_PROGRAM_CACHE = {}


def _get_program(nsh, usteps, n_iters, repeat=1):
    key = (nsh, usteps, n_iters, repeat)
    if key not in _PROGRAM_CACHE:
        _PROGRAM_CACHE[key] = _build_program(nsh, usteps, n_iters, repeat)
    return _PROGRAM_CACHE[key]


def pick_loop(nsh):
    """Return (usteps, n_iters) with usteps * n_iters = n_steps - 1."""
    n_steps = nsh // (2 * FD)
    m = n_steps - 1
    for u in (7, 9, 21, 3, 63):
        if m % u == 0 and m // u >= 1:
            return u, m // u
    return m, 1


def _input_maps(uv, w_in, b_in, w_hid, b_hid, w_out, b_out):
    n = uv.shape[0]
    nsh = n // N_CORES
    bp, bn, bo = _effective_params(w_in, b_in, w_hid, b_hid, w_out, b_out)
    base = {
        "WIN": np.ascontiguousarray(w_in.astype(np.float32)),
        "WH": np.ascontiguousarray(w_hid.reshape(8, H, H).astype(np.float32)),
        "WO": np.ascontiguousarray(w_out.astype(np.float32)),
        "IDE": np.eye(D, dtype=np.float32),
        "BP": bp,
        "BN": bn,
        "BO": bo,
    }
    in_maps = []
    for c in range(N_CORES):
        shard = uv[c * nsh:(c + 1) * nsh]
        m = dict(base)
        m["uvT"] = np.ascontiguousarray(shard.T.astype(np.float32))
        in_maps.append(m)
    return in_maps, nsh


def kernel(uv, w_in, b_in, w_hid, b_hid, w_out, b_out):
    n = uv.shape[0]
    nsh = n // N_CORES
    usteps, n_iters = pick_loop(nsh)
    in_maps, _ = _input_maps(uv, w_in, b_in, w_hid, b_hid, w_out, b_out)
    nc = _get_program(nsh, usteps, n_iters)
    res = run_bass_kernel_spmd(nc, in_maps, core_ids=list(range(N_CORES)))
    outs = [res.results[c]["outT"].T for c in range(N_CORES)]
    return np.ascontiguousarray(np.concatenate(outs, axis=0)).astype(np.float32)


# revision 23
# speedup vs baseline: 1.0306x; 1.0306x over previous
"""Trainium2 Bass kernel for a chain of 2 invertible-ResNet blocks
(dense MLP 2->256, 4x 256->256, 256->2, ELU, residual) over 1M points.

Strategy: pure data parallel over 8 NeuronCores; points transposed to
[2, N] on host so activations live as [256, FD] tiles (features on
partitions, points on the free dim).  Matmuls run in float32r (full PE
rate).  ELU is computed in 2 instructions per tile:
    ACT:  e  = Exp(y + b_eff)          (PSUM -> SBUF, bias fused)
    DVE:  h'' = max(y, -b_eff) + min(e, 1)   (one custom fused op)
with the resulting constant shift (b_eff - 1) folded into the next
layer's effective bias (precomputed on host).

v3: FD=1024 two-PSUM-bank m-tile units (one ACT + one DVE instruction
per [128, 1024] m-tile halves the per-op fixed overhead on the two
bottleneck engines; DVE at 0.96 GHz 1x is the binding engine at
~23.8 us busy per 1024-point chunk, ACT ~23.2, PE ~18.1) + block-
granular software pipelining: stream B runs one iResNet block (5 ELU
layers) behind stream A, layer-interleaved in emission order, so the
serial block boundary (out-proj matmuls -> ACT evacuation -> next
block's input matmuls) of either stream is always covered by ~5 layers
of dense MM/ACT/DVE work from the other stream.  Step order:
    zip(A.block0(c), B.block1(c-1)) ; zip(A.block1(c), B.block0(c+1))
x1 state lives in persistent SBUF tiles carried across For_i
iterations; prologue/epilogue peel the pipeline edges.  usteps=7 steps
(14 chunks, ~1900 instructions) per For_i iteration balances the
per-iteration all-engine barrier against sequencer instruction-fetch
pressure (bodies over ~2500 instructions degrade; fully unrolled is
~4% slower).
"""

import numpy as np

import concourse.bass as bass
import concourse.tile as tile
from concourse import bacc, mybir
from concourse.bass_utils import run_bass_kernel_spmd
from concourse.dve_spec import Spec, Src0, Src1, C0, C1, maxx, minn
import concourse.dve_ops as dve_ops
from concourse.dve_ops import DveOp

F32 = mybir.dt.float32
F32R = mybir.dt.float32r

NUM_NODES = 2
H = 256
L = 4
D = 2
N_CORES = 8

FD = 1024          # points per chunk (free dim; m-tile = 2 PSUM banks)
HB = 512           # matmul column-half (one PSUM bank)


def _register_elu_tail():
    name = "ELU_TAIL_ANT"
    for op in dve_ops.OPS:
        if op.name == name:
            return op
    op = DveOp(
        name,
        Spec(
            body=maxx(Src0, C0) + minn(Src1, C1),
            reference=lambda in0, in1, s0, s1, imm2: (
                np.maximum(in0.astype(np.float32), s0)
                + np.minimum(in1.astype(np.float32), s1)
            ),
        ),
        subdim=False,
        uops_sha={"v3": "b9e41bc1a54edf6f", "v4": "2155f01abd9df135"},
    )
    dve_ops.OPS.append(op)
    dve_ops._SUB_OPCODE_FOR_NAME[name] = (
        dve_ops._CUSTOM_DVE_ROW_BASE + len(dve_ops.OPS) - 1
    )
    dve_ops.CUSTOM_DVE_SPECS[name] = op.spec
    return op


def _effective_params(w_in, b_in, w_hid, b_hid, w_out, b_out):
    """Fold the ELU-tail constant shifts into effective biases (float64).

    Unfused form: x1 is materialized, so block1's input bias is the true
    b_in[1]; bo0 rides the x1 evacuation and bo1 the final output.
    """
    w_hid = w_hid.astype(np.float64)
    b_hid = b_hid.astype(np.float64)
    w_out = w_out.astype(np.float64)
    b_out = b_out.astype(np.float64)
    b_in = b_in.astype(np.float64)

    b_eff = np.zeros((2 * (1 + L), H))          # per ELU layer
    b_eff[0] = b_in[0]
    c = b_eff[0] - 1.0
    for l in range(L):
        b_eff[1 + l] = b_hid[0, l] + c @ w_hid[0, l]
        c = b_eff[1 + l] - 1.0
    bo0 = b_out[0] + c @ w_out[0]               # [2] -> x1 evac bias
    b_eff[5] = b_in[1]
    c = b_eff[5] - 1.0
    for l in range(L):
        b_eff[6 + l] = b_hid[1, l] + c @ w_hid[1, l]
        c = b_eff[6 + l] - 1.0
    bo1 = b_out[1] + c @ w_out[1]               # [2] -> final out bias

    bp = np.zeros((128, 20), np.float32)
    bn = np.zeros((128, 20), np.float32)
    for j in range(10):
        for m in range(2):
            col = b_eff[j, m * 128:(m + 1) * 128]
            bp[:, j * 2 + m] = col.astype(np.float32)
            bn[:, j * 2 + m] = (-col).astype(np.float32)
    bo = np.stack([bo0, bo1], axis=1).astype(np.float32)   # [2, 2]
    return bp, bn, bo


def _build_program(nsh, usteps, n_iters, repeat=1):
    """Build the SPMD Bass program for one core processing `nsh` points.

    nsh = (1 + usteps * n_iters) * 2 * FD  total chunks; each step is a
    pair of chunks (stream A even, stream B odd).  When n_iters > 1 a
    hardware For_i loop runs `usteps` steps per iteration; step 0 and
    the last B.block1 are peeled into prologue/epilogue.  `repeat`
    re-runs the whole pass (benchmarking aid; output identical).
    """
    ELU_TAIL = _register_elu_tail()
    nc = bacc.Bacc("TRN2", target_bir_lowering=False, debug=False,
                   num_devices=N_CORES)

    uvT = nc.declare_dram_parameter("uvT", [D, nsh], F32, isOutput=False).ap()
    WIN = nc.declare_dram_parameter("WIN", [2, D, H], F32, isOutput=False).ap()
    WH = nc.declare_dram_parameter("WH", [8, H, H], F32, isOutput=False).ap()
    WO = nc.declare_dram_parameter("WO", [2, H, D], F32, isOutput=False).ap()
    IDE = nc.declare_dram_parameter("IDE", [D, D], F32, isOutput=False).ap()
    BP = nc.declare_dram_parameter("BP", [128, 20], F32, isOutput=False).ap()
    BN = nc.declare_dram_parameter("BN", [128, 20], F32, isOutput=False).ap()
    BO = nc.declare_dram_parameter("BO", [D, 2], F32, isOutput=False).ap()
    outT = nc.declare_dram_parameter("outT", [D, nsh], F32, isOutput=True).ap()

    with tile.TileContext(nc) as tc:
        with (
            tc.tile_pool(name="wpool", bufs=1) as wp,
            tc.tile_pool(name="xpool", bufs=3) as xp,
            tc.tile_pool(name="epool", bufs=2) as ep,
            tc.tile_pool(name="hpool", bufs=2) as hp,
            tc.tile_pool(name="opool", bufs=2) as op,
            tc.tile_pool(name="ypool", bufs=2, space="PSUM") as yp,
        ):
            # ---- persistent weights/biases (loaded once) ----
            win = [wp.tile([D, H], F32R, tag=f"win{i}", name=f"win{i}") for i in range(2)]
            for i in range(2):
                nc.gpsimd.dma_start(out=win[i], in_=WIN[i])
            wh = [[wp.tile([128, H], F32R, tag=f"wh{j}k{k}", name=f"wh{j}k{k}") for k in range(2)]
                  for j in range(8)]
            for j in range(8):
                for k in range(2):
                    nc.gpsimd.dma_start(out=wh[j][k],
                                        in_=WH[j, k * 128:(k + 1) * 128, :])
            wo = [[wp.tile([128, D], F32R, tag=f"wo{i}k{k}", name=f"wo{i}k{k}") for k in range(2)]
                  for i in range(2)]
            for i in range(2):
                for k in range(2):
                    nc.gpsimd.dma_start(out=wo[i][k],
                                        in_=WO[i, k * 128:(k + 1) * 128, :])
            ide = wp.tile([D, D], F32R, tag="ide")
            nc.gpsimd.dma_start(out=ide, in_=IDE)
            bp = wp.tile([128, 20], F32, tag="bp")
            nc.gpsimd.dma_start(out=bp, in_=BP)
            bn = wp.tile([128, 20], F32, tag="bn")
            nc.gpsimd.dma_start(out=bn, in_=BN)
            bo = wp.tile([D, 2], F32, tag="bo")
            nc.gpsimd.dma_start(out=bo, in_=BO)
            # persistent x1 state per stream (carried across steps)
            x1 = [wp.tile([D, FD], F32R, tag=f"x1s{s}", name=f"x1s{s}")
                  for s in range(2)]

            def load_x0(s, sl):
                x0 = xp.tile([D, FD], F32R, name=f"x0s{s}", tag=f"x0s{s}")
                nc.gpsimd.dma_start(out=x0, in_=uvT[:, sl])
                return x0

            def block_items(s, i, xin, out_sl=None):
                """Emission items (one per layer + out-proj) for block i of
                stream s.  Returned closures are called in order, possibly
                interleaved with another stream's items.

                xin: [D, FD] SBUF input (x0 for block0, x1 for block1).
                block0 evacuates into the persistent x1[s]; block1 into a
                fresh xo tile, DMA'd to outT[:, out_sl]."""
                st = {"h": [None, None]}

                def layer_item(jj):
                    j = 5 * i + jj
                    h = st["h"]
                    newh = [None, None]
                    for m in range(2):
                        mcs = slice(m * 128, (m + 1) * 128)
                        y = yp.tile([128, FD], F32, name=f"ys{s}m{m}",
                                    tag=f"ys{s}")
                        for hf in range(2):
                            cs = slice(hf * HB, (hf + 1) * HB)
                            if jj == 0:
                                nc.tensor.matmul(y[:, cs], win[i][:, mcs],
                                                 xin[:, cs],
                                                 start=True, stop=True)
                            else:
                                jh = 4 * i + jj - 1
                                nc.tensor.matmul(y[:, cs], wh[jh][0][:, mcs],
                                                 h[0][:, cs],
                                                 start=True, stop=False)
                                nc.tensor.matmul(y[:, cs], wh[jh][1][:, mcs],
                                                 h[1][:, cs],
                                                 start=False, stop=True)
                        col = j * 2 + m
                        e = ep.tile([128, FD], F32, name=f"es{s}m{m}",
                                    tag=f"es{s}m{m}")
                        nc.scalar.activation(
                            e, y, mybir.ActivationFunctionType.Exp,
                            bias=bp[:, col:col + 1])
                        hn = hp.tile([128, FD], F32R, name=f"hs{s}m{m}",
                                     tag=f"hs{s}m{m}")
                        nc.vector._custom_dve(ELU_TAIL, out=hn, in0=y, in1=e,
                                              s0=bn[:, col:col + 1], s1=1.0)
                        newh[m] = hn
                    st["h"] = newh

                def out_item():
                    h = st["h"]
                    if i == 0:
                        dst = x1[s]
                    else:
                        dst = op.tile([D, FD], F32, name=f"xos{s}",
                                      tag=f"xos{s}")
                    for hf in range(2):
                        cs = slice(hf * HB, (hf + 1) * HB)
                        yo = yp.tile([D, HB], F32, name=f"yo{i}s{s}h{hf}",
                                     tag=f"ys{s}")
                        nc.tensor.matmul(yo, ide, xin[:, cs],
                                         start=True, stop=False)
                        nc.tensor.matmul(yo, wo[i][0], h[0][:, cs],
                                         start=False, stop=False)
                        nc.tensor.matmul(yo, wo[i][1], h[1][:, cs],
                                         start=False, stop=True)
                        nc.scalar.activation(
                            dst[:, cs], yo,
                            mybir.ActivationFunctionType.Identity,
                            bias=bo[:, i:i + 1])
                    if i == 1:
                        nc.sync.dma_start(out=outT[:, out_sl], in_=dst)

                return [lambda jj=jj: layer_item(jj) for jj in range(5)] + \
                    [out_item]

            def mk(base):
                return (slice(base, base + FD) if isinstance(base, int)
                        else bass.ds(base, FD))

            def interleave(xs, ys):
                for t in range(max(len(xs), len(ys or []))):
                    if t < len(xs):
                        xs[t]()
                    if ys and t < len(ys):
                        ys[t]()

            def step(abase, first=False):
                """One pipeline step.  abase = A-chunk base offset (int or
                register expr); B of this step is abase + FD; B's block1 of
                the PREVIOUS step (chunk abase - FD) is emitted here,
                layer-interleaved with A's, so B lags A by one block and
                every block boundary is covered by the other stream."""
                asl = mk(abase)
                x0a = load_x0(0, asl)
                bi = (None if first
                      else block_items(1, 1, x1[1], mk(abase - FD)))
                interleave(block_items(0, 0, x0a), bi)     # A.b0 || B.b1'
                x0b = load_x0(1, mk(abase + FD))
                interleave(block_items(0, 1, x1[0], asl),  # A.b1 || B.b0
                           block_items(1, 0, x0b))

            n_steps = nsh // (2 * FD)
            assert n_steps == 1 + usteps * n_iters, (nsh, usteps, n_iters)
            for _rep in range(repeat):
                # prologue: step 0 (no lagged B.block1 yet)
                step(0, first=True)
                if n_iters == 1:
                    for st in range(1, n_steps):
                        step(st * 2 * FD)
                else:
                    stride = 2 * FD * usteps
                    with tc.For_i(2 * FD, 2 * FD + n_iters * stride, stride,
                                  hint_engines=(mybir.EngineType.PE,)) as it:
                        for t in range(usteps):
                            step(it + t * 2 * FD)
                # epilogue: last B.block1
                interleave(block_items(
                    1, 1, x1[1],
                    slice((2 * n_steps - 1) * FD, 2 * n_steps * FD)), None)

    nc.finalize()
    return nc


_PROGRAM_CACHE = {}


def _get_program(nsh, usteps, n_iters, repeat=1):
    key = (nsh, usteps, n_iters, repeat)
    if key not in _PROGRAM_CACHE:
        _PROGRAM_CACHE[key] = _build_program(nsh, usteps, n_iters, repeat)
    return _PROGRAM_CACHE[key]


def pick_loop(nsh):
    """Return (usteps, n_iters) with usteps * n_iters = n_steps - 1."""
    n_steps = nsh // (2 * FD)
    m = n_steps - 1
    for u in (7, 9, 21, 3, 63):
        if m % u == 0 and m // u >= 1:
            return u, m // u
    return m, 1


def _input_maps(uv, w_in, b_in, w_hid, b_hid, w_out, b_out):
    n = uv.shape[0]
    nsh = n // N_CORES
    bp, bn, bo = _effective_params(w_in, b_in, w_hid, b_hid, w_out, b_out)
    base = {
        "WIN": np.ascontiguousarray(w_in.astype(np.float32)),
        "WH": np.ascontiguousarray(w_hid.reshape(8, H, H).astype(np.float32)),
        "WO": np.ascontiguousarray(w_out.astype(np.float32)),
        "IDE": np.eye(D, dtype=np.float32),
        "BP": bp,
        "BN": bn,
        "BO": bo,
    }
    in_maps = []
    for c in range(N_CORES):
        shard = uv[c * nsh:(c + 1) * nsh]
        m = dict(base)
        m["uvT"] = np.ascontiguousarray(shard.T.astype(np.float32))
        in_maps.append(m)
    return in_maps, nsh


def kernel(uv, w_in, b_in, w_hid, b_hid, w_out, b_out):
    n = uv.shape[0]
    nsh = n // N_CORES
    usteps, n_iters = pick_loop(nsh)
    in_maps, _ = _input_maps(uv, w_in, b_in, w_hid, b_hid, w_out, b_out)
    nc = _get_program(nsh, usteps, n_iters)
    res = run_bass_kernel_spmd(nc, in_maps, core_ids=list(range(N_CORES)))
    outs = [res.results[c]["outT"].T for c in range(N_CORES)]
    return np.ascontiguousarray(np.concatenate(outs, axis=0)).astype(np.float32)
